# revision 11
# baseline (speedup 1.0000x reference)
"""Trainium2 Bass kernel for nn_DE3 (histogram_binning + entropy).

Full input: img [16, 2048, 2048] f32 with values in [0, 256).
reference = B * (8 - res), res = -sum p log2 p, p = bincount(floor(img)) / (H*W).

Strategy (8 NeuronCores, data parallel):
  - Split the 64Mi elements into 8 shards of 8Mi (one per core).
  - Per core, compute the 2-D cumulative-count matrix
        J[i, j] = #{e : hi_e >= i  AND  lo_e >= j},   i, j in [0, 16)
    where idx = floor(x) = 16*hi + lo. J is accumulated on the PE
    (one [128,16]x[128,16] bf16 matmul per 128 elements into a single
    PSUM tile). The hi/lo "ladders" (is_ge one-hot cumulants) are built
    on DVE/ACT/GPSIMD at a few cycles per element.
  - Host: sum J over cores, 2-D finite difference -> 256-bin counts,
    then the trivial entropy epilogue.
"""

import numpy as np

import concourse.bass as bass
import concourse.mybir as mybir
from concourse.tile import TileContext
from concourse.bass_utils import run_bass_kernel_spmd

P = 128          # SBUF partitions
F = 512          # free-dim elements per tile
N_CORES = 8

_BIG = float(3 * 2**22)  # 1.5*2^23: keeps t in [2^23, 2^24) where ulp = 1

_MAX_WAITS = 1  # this walrus build supports at most 1 sync-wait per instruction


def _split_excess_waits(nc):
    """Walrus in this container rejects instructions with >2 sync-wait
    commands (Tile's tail drain can carry many). Move excess waits onto
    same-engine NoOp instructions inserted just before the offender."""
    n_split = 0
    for f in nc.m.functions:
        for bb in f.blocks:
            out = []
            for ins in bb.instructions:
                si = getattr(ins, "sync_info", None)
                waits = list(si.on_wait) if si is not None and si.on_wait else []
                if len(waits) > _MAX_WAITS:
                    extra, keep = waits[:-_MAX_WAITS], waits[-_MAX_WAITS:]
                    for ci in range(0, len(extra), _MAX_WAITS):
                        chunk = extra[ci : ci + _MAX_WAITS]
                        nop = mybir.InstNoOp(
                            name=f"{ins.name}-wsplit{ci}",
                            engine=ins.engine,
                            sync_info=mybir.SyncInfo(on_wait=chunk, on_update=[]),
                        )
                        out.append(nop)
                        n_split += 1
                    si.on_wait = keep
                out.append(ins)
            bb.instructions = out
    return n_split


def build_nc(n_tiles: int, debug: bool = False):
    """Build the Bass kernel: input [n_tiles*P, F] f32 -> output J [16,16] f32."""
    nc = bass.Bass()
    # const AP for the ACT-engine bias (-2^23), mirroring Bass's own init
    _ct = nc.alloc_sbuf_tensor("const-neg-big", [128, 1], mybir.dt.float32)
    nc.gpsimd.memset(_ct.ap(), -_BIG)
    nc.const_aps.aps[(mybir.dt.float32, -_BIG)] = _ct.ap()
    nc.all_engine_barrier()
    x_in = nc.declare_dram_parameter(
        "x", [n_tiles * P, F], mybir.dt.float32, isOutput=False
    )
    j_out = nc.declare_dram_parameter("j", [16, 16], mybir.dt.float32, isOutput=True)
    if debug:
        dbg_hi = nc.declare_dram_parameter("dbg_hi", [P, F], mybir.dt.float32, isOutput=True)
        dbg_lo = nc.declare_dram_parameter("dbg_lo", [P, F], mybir.dt.float32, isOutput=True)
        dbg_lhi = nc.declare_dram_parameter("dbg_lhi", [P, 16 * F], mybir.dt.float32, isOutput=True)
        dbg_llo = nc.declare_dram_parameter("dbg_llo", [P, 16 * F], mybir.dt.float32, isOutput=True)

    dt = mybir.dt
    op = mybir.AluOpType

    with TileContext(nc) as tc:
        with (
            tc.tile_pool(name="data", bufs=3) as dpool,
            tc.tile_pool(name="lad", bufs=2) as lpool,
            tc.tile_pool(name="psum", bufs=1, space="PSUM") as ppool,
            tc.tile_pool(name="outp", bufs=1) as opool,
        ):
            jt = ppool.tile([16, 16], dt.float32)
            for it in range(n_tiles):
                x = dpool.tile([P, F], dt.float32, tag="x")
                nc.sync.dma_start(out=x[:], in_=x_in[it * P : (it + 1) * P, :])
                # xb = x - 8 (exact); carries the -0.5 through /16 for the
                # floor-by-round trick, since BIG-0.5 is not representable.
                xb = dpool.tile([P, F], dt.float32, tag="xb")
                nc.vector.tensor_scalar(
                    out=xb[:], in0=x[:], scalar1=-8.0, scalar2=None, op0=op.add
                )
                # t = xb/16 + 1.5*2^23 = (x/16 - 0.5) + BIG -> RN: BIG + floor(x/16)
                t = dpool.tile([P, F], dt.float32, tag="t")
                nc.vector.tensor_scalar(
                    out=t[:], in0=xb[:], scalar1=1.0 / 16.0, scalar2=_BIG,
                    op0=op.mult, op1=op.add,
                )
                # hi = t - 1.5*2^23 in [0,16], exact small int -> bf16 (ACT engine)
                hi8 = dpool.tile([P, F], dt.bfloat16, tag="hi8")
                nc.scalar.add(hi8[:], t[:], -_BIG)
                # yb = xb - 16*hi = (x - 16*hi) - 8  in [-8, 8)  (exact)
                yb = dpool.tile([P, F], dt.float32, tag="yb")
                nc.vector.scalar_tensor_tensor(
                    out=yb[:], in0=hi8[:], scalar=-16.0, in1=xb[:],
                    op0=op.mult, op1=op.add,
                )
                # u = (yb + 7.5) + BIG = (lo_frac - 0.5) + BIG -> RN: BIG + lo
                u = dpool.tile([P, F], dt.float32, tag="u")
                nc.vector.tensor_scalar(
                    out=u[:], in0=yb[:], scalar1=7.5, scalar2=_BIG,
                    op0=op.add, op1=op.add,
                )
                # lo = u - 1.5*2^23 in [0,16] exact -> bf16 (ACT engine)
                lo8 = dpool.tile([P, F], dt.bfloat16, tag="lo8")
                nc.scalar.add(lo8[:], u[:], -_BIG)

                # ladders: lhi[p, i, f] = (hi >= i), llo[p, j, f] = (lo >= j)
                lhi = lpool.tile([P, 16, F], dt.bfloat16, tag="lhi")
                llo = lpool.tile([P, 16, F], dt.bfloat16, tag="llo")
                for j in range(16):
                    nc.vector.tensor_scalar(
                        out=lhi[:, j, :], in0=hi8[:], scalar1=float(j), scalar2=None,
                        op0=op.is_ge,
                    )
                    nc.vector.tensor_scalar(
                        out=llo[:, j, :], in0=lo8[:], scalar1=float(j), scalar2=None,
                        op0=op.is_ge,
                    )
                if debug and it == 0:
                    fhi = dpool.tile([P, F], dt.float32, tag="fhi")
                    nc.vector.tensor_copy(out=fhi[:], in_=hi8[:])
                    nc.sync.dma_start(out=dbg_hi[:], in_=fhi[:])
                    flo = dpool.tile([P, F], dt.float32, tag="flo")
                    nc.vector.tensor_copy(out=flo[:], in_=lo8[:])
                    nc.sync.dma_start(out=dbg_lo[:], in_=flo[:])
                    flh = lpool.tile([P, 16 * F], dt.float32, tag="flh")
                    nc.vector.tensor_copy(out=flh[:], in_=lhi[:].rearrange('p a b -> p (a b)'))
                    nc.sync.dma_start(out=dbg_lhi[:], in_=flh[:])
                    fll = lpool.tile([P, 16 * F], dt.float32, tag="fll")
                    nc.vector.tensor_copy(out=fll[:], in_=llo[:].rearrange('p a b -> p (a b)'))
                    nc.sync.dma_start(out=dbg_llo[:], in_=fll[:])
                # PE: accumulate J += lhi_c^T @ llo_c for each 128-elem column c
                for c in range(F):
                    nc.tensor.matmul(
                        jt[:],
                        lhsT=lhi[:, :, c],
                        rhs=llo[:, :, c],
                        start=(it == 0 and c == 0),
                        stop=(it == n_tiles - 1 and c == F - 1),
                    )
            jsb = opool.tile([16, 16], dt.float32)
            nc.vector.tensor_copy(out=jsb[:], in_=jt[:])
            nc.sync.dma_start(out=j_out[:], in_=jsb[:])
    _split_excess_waits(nc)
    return nc


def _counts_from_J(J: np.ndarray) -> np.ndarray:
    """J [16,16] cumulative -> counts [256] (bin = 16*hi + lo)."""
    Jp = np.zeros((17, 17), dtype=np.float64)
    Jp[:16, :16] = J
    A = Jp[:16, :] - Jp[1:, :]
    c2 = A[:, :16] - A[:, 1:]
    return c2.reshape(256)


def kernel(img: np.ndarray) -> np.ndarray:
    img = np.asarray(img, dtype=np.float32)
    B, H, W = img.shape
    flat = img.reshape(-1)
    n = flat.size
    assert n % (N_CORES * P * F) == 0
    shard = n // N_CORES
    n_tiles = shard // (P * F)

    nc = build_nc(n_tiles)
    in_maps = [
        {"x": flat[i * shard : (i + 1) * shard].reshape(n_tiles * P, F)}
        for i in range(N_CORES)
    ]
    res = run_bass_kernel_spmd(nc, in_maps, list(range(N_CORES)))
    J = np.zeros((16, 16), dtype=np.float64)
    for r in res.results:
        J += np.asarray(r["j"], dtype=np.float64)

    counts = _counts_from_J(J)
    temp = float(H * W)
    p = counts / temp
    with np.errstate(divide="ignore", invalid="ignore"):
        terms = np.where(p > 0, p * np.log2(np.where(p > 0, p, 1.0)), 0.0)
    ent = -terms.sum()
    out = np.float32(B * (8.0 - ent))
    return np.asarray(out, dtype=np.float32)
